# revision 58
# baseline (speedup 1.0000x reference)
"""BigBird sparse attention on 8 Trainium2 NeuronCores.

Sharding: 16 heads across 8 cores (2 heads/core, both batches per core).

Per-core pipeline (layouts chosen to minimize PE matmul count, keep PE
streams long so the p-state ramps to 2.4 GHz, and keep DMA dispatch off the
critical path):

  stage A  row-major fused QKV projection: per 128-row block, stationary =
           x-block [128dc,128r], moving = wqkv [128dc,384] -> psum [128,384],
           bias-add/copy into q-rows / kv-rows bf16 (DVE+Scalar).  KV row
           slices DMA directly to kv_stage DRAM.  x loads ride the Activation
           HWDGE queue; stores ride the SP queue (no head-of-line blocking).
  gathers  fused KV row gather (row-mode SWDGE, 1024B descriptors, 6 calls
           spread over 4 SWDGE queues), order (t, m, i) so chunk (t,m) holds
           key ri[t*128+i, m] at partition i.
  qT/kT    PE transposes (bf16 psum) + Scalar/DVE copies.
  QK       column layout scores^T[j,i]: stationary = kT j-block [64,128],
           moving = 384+2 contiguous q columns (3 i-blocks + global rows);
           j-block 0 streams all 2048 i columns.  The BigBird mask is applied
           as a second accumulating matmul (-30 * complement-mask with a
           -30*identity stationary), so exp (Scalar) reads psum directly and
           no element-wise mask pass exists.
  rand     random-key scores as DVE row-wise dot products (ksel . q-rows),
           exp+dedup weights, weighted V row-sum per i-block whose transpose
           (regular bf16 matmul vs identity) accumulates into the PV psum;
           the softmax denominator rides as column 64.
  PV       stationary = vaug j-block [128,65] (V rows + ones rider), moving =
           exp'd probability tiles; single [65,2048] psum accumulator per
           (b,h); row 64 = denominator.  Normalize via K=1 broadcast matmul +
           DVE fast-reciprocal/multiply into per-head hout [64, r].
  stage D  partial o-projection accumulating over the 2 heads (K=64);
           bf16 partial outputs summed on host.
"""

import math
import numpy as np

# ---------------------------------------------------------------- constants
B = 2
S = 2048
D = 1024
H = 16
HD = 64
NG = 2          # num global tokens
NR = 3          # num random keys per row
WIN = 3         # window half-width

N_CORES = 8
HPC = H // N_CORES          # heads per core = 2
HD2 = HPC * HD              # 128 = head-dim slice per core
R = B * S                   # 4096 flattened rows
NT = S // 128               # 16 i-blocks per (b, h)
NRB = 32                    # row blocks (R / 128)
NCH = NT * NR               # 48 gather chunks of 128
NIDX = NCH * 128            # 6144 gather indices
KVROW = 512                 # kv_stage row elements (2 batches x (K|V) x 128)
NEG = -30.0                 # mask penalty (exp(-30) ~ 1e-13)

INV_SQRT_HD = 1.0 / math.sqrt(float(HD))


# i-column range served by k-block jb (band + global rows); jb 0 serves all
def _serve(jb):
    if jb == 0:
        return 0, 2048, False
    i0 = (jb - 1) * 128
    w = min(384, 2048 - i0)
    return i0, w, (i0 > 0)   # extra 2 cols for global rows when i0 > 0


def _mask_kind(jb):
    if jb == 0:
        return 0
    if jb == 1:
        return 1
    if jb == 15:
        return 3
    return 2


# ---------------------------------------------------------------- host prep
def _host_masks():
    """COMPLEMENT structural (window|global) masks, transposed [j, i]."""
    import ml_dtypes

    j = np.arange(S)[:, None]
    i = np.arange(S)[None, :]
    Dm = (i < NG) | (j < NG) | (np.abs(i - j) <= WIN)   # [j, i] structural

    c0 = NEG * (~Dm[0:128, :]).astype(np.float32)                 # [128,2048]
    c1 = NEG * (~Dm[128:256, 0:384]).astype(np.float32)           # [128, 384]
    jr = np.arange(128)[:, None]
    ic = np.arange(384)[None, :]
    cI = NEG * (~(np.abs(ic - 128 - jr) <= WIN)).astype(np.float32)
    cI = np.concatenate([cI, np.zeros((128, 2), np.float32)], axis=1)
    c15 = np.concatenate([cI[:, 0:256], np.zeros((128, 2), np.float32)],
                         axis=1)                                   # [128, 258]
    bf = ml_dtypes.bfloat16
    return c0.astype(bf), c1.astype(bf), cI.astype(bf), c15.astype(bf)


def _host_idx_w(random_indices):
    """Gather indices (t, m, i order) and dedup weights w [128, 48]."""
    import ml_dtypes

    ri = np.asarray(random_indices).astype(np.int64)   # [S, NR]
    n = np.arange(NIDX)
    t_of = n // (NR * 128)
    m_of = (n // 128) % NR
    p_of = n % 128
    i_of = t_of * 128 + p_of
    j_of = ri[i_of, m_of]
    a16 = np.zeros((16, NIDX // 16), dtype=np.int16)
    a16[n % 16, n // 16] = j_of.astype(np.int16)
    gidx = np.tile(a16, (8, 1))                        # [128, NIDX//16]

    # w[p, (t, m)]: 0 if structurally covered / global row / duplicate
    w = np.ones((128, NCH), dtype=np.float32)
    for t in range(NT):
        for m in range(NR):
            c = t * NR + m
            for p in range(128):
                i = t * 128 + p
                r = ri[i, m]
                if i < NG or r < NG or abs(i - r) <= WIN:
                    w[p, c] = 0.0
                elif any(ri[i, mm] == r for mm in range(m)):
                    w[p, c] = 0.0
    return gidx, w.astype(ml_dtypes.bfloat16)


def make_in_maps(inputs):
    """Full inputs -> list of 8 per-core input dicts."""
    import ml_dtypes
    bf = ml_dtypes.bfloat16

    x = np.asarray(inputs["x"], dtype=np.float32)
    ri = np.asarray(inputs["random_indices"])
    q_w = np.asarray(inputs["q_w"], dtype=np.float32) * INV_SQRT_HD
    k_w = np.asarray(inputs["k_w"], dtype=np.float32)
    v_w = np.asarray(inputs["v_w"], dtype=np.float32)
    o_w = np.asarray(inputs["o_w"], dtype=np.float32)
    q_b = np.asarray(inputs["q_b"], dtype=np.float32) * INV_SQRT_HD
    k_b = np.asarray(inputs["k_b"], dtype=np.float32)
    v_b = np.asarray(inputs["v_b"], dtype=np.float32)

    # x blocks: xprep[p, rb, dc, rl] = xT[dc*128+p, rb*128+rl]
    xT = np.ascontiguousarray(x.reshape(R, D).T)                  # [D, R]
    xprep = np.ascontiguousarray(
        xT.reshape(8, 128, NRB, 128).transpose(1, 2, 0, 3)).astype(bf)

    mc0, mc1, mcI, mc15 = _host_masks()
    gidx, wrand = _host_idx_w(ri)

    in_maps = []
    for c in range(N_CORES):
        sl = slice(HD2 * c, HD2 * (c + 1))
        wqkv = np.concatenate(
            [q_w[sl, :].T, k_w[sl, :].T, v_w[sl, :].T], axis=1)   # [D, 384]
        wqkv = np.ascontiguousarray(
            wqkv.reshape(8, 128, 384).transpose(1, 0, 2)).astype(bf)
        bias = np.concatenate([q_b[sl], k_b[sl], v_b[sl]])        # [384]
        biasb = np.ascontiguousarray(
            np.broadcast_to(bias[None, :], (128, 384))).copy()
        in_maps.append({
            "xprep": xprep,
            "wqkv": wqkv,
            "biasb": biasb,
            "mc0": mc0, "mc1": mc1, "mcI": mcI, "mc15": mc15,
            "wrand": wrand,
            "gidx": gidx,
            "wo2": np.ascontiguousarray(o_w[:, sl].T).astype(bf),  # [128, D]
        })
    return in_maps


# ---------------------------------------------------------------- device IR
def build_kernel(bias_zero=True):
    import os
    import concourse.tile as tile
    from concourse import bacc, mybir

    nc = bacc.Bacc("TRN2", target_bir_lowering=False, debug=False,
                   num_swdge_queues=int(os.environ.get("K_GQ", "4")))
    f32 = mybir.dt.float32
    bf16 = mybir.dt.bfloat16
    i16 = mybir.dt.int16

    t_ = dict(
        xprep=nc.dram_tensor("xprep", [128, NRB, 8, 128], bf16,
                             kind="ExternalInput").ap(),
        wqkv=nc.dram_tensor("wqkv", [128, 8, 384], bf16,
                            kind="ExternalInput").ap(),
        biasb=nc.dram_tensor("biasb", [128, 384], f32,
                             kind="ExternalInput").ap(),
        mc0=nc.dram_tensor("mc0", [128, 2048], bf16,
                           kind="ExternalInput").ap(),
        mc1=nc.dram_tensor("mc1", [128, 384], bf16,
                           kind="ExternalInput").ap(),
        mcI=nc.dram_tensor("mcI", [128, 386], bf16,
                           kind="ExternalInput").ap(),
        mc15=nc.dram_tensor("mc15", [128, 258], bf16,
                            kind="ExternalInput").ap(),
        wrand=nc.dram_tensor("wrand", [128, NCH], bf16,
                             kind="ExternalInput").ap(),
        gidx=nc.dram_tensor("gidx", [128, NIDX // 16], i16,
                            kind="ExternalInput").ap(),
        wo2=nc.dram_tensor("wo2", [HD2, D], bf16, kind="ExternalInput").ap(),
        out=nc.dram_tensor("out_part", [R, D], bf16,
                           kind="ExternalOutput").ap(),
        kv_stage=nc.dram_tensor("kv_stage", [S, KVROW], bf16).ap(),
    )
    if os.environ.get("K_DEBUG", "0") == "1":
        for nm, shp in (("dbg_qrows", [128, NRB, 128]),
                        ("dbg_kvrows", [128, NRB, 256]),
                        ("dbg_qT", [128, R]), ("dbg_kT", [128, R]),
                        ("dbg_ksel", [128, NCH, KVROW]),
                        ("dbg_pj", [128, 8192]),
                        ("dbg_wsumf", [128, NT, HD + 1]),
                        ("dbg_den", [HD + 1, 2048]),
                        ("dbg_rinv", [HD, 2048]),
                        ("dbg_hout0", [HD, R]), ("dbg_hout1", [HD, R])):
            dt = mybir.dt.float32 if nm in ("dbg_wsumf", "dbg_rinv") \
                else bf16
            t_[nm] = nc.dram_tensor(nm, shp, dt, kind="ExternalOutput").ap()

    with tile.TileContext(nc) as tc:
        _build_tc(nc, tc, t_, bias_zero)
    nc.compile()
    return nc


def _build_tc(nc, tc, t_, bias_zero):
    import os
    from contextlib import ExitStack

    import concourse.bass as bass
    from concourse import masks as cmasks, mybir

    RECIP_FAST = os.environ.get("K_RECIP", "fast") == "fast"
    TRANS_PE = os.environ.get("K_TRANS", "pe") == "pe"
    MASK_MM = os.environ.get("K_MASKMM", "1") == "1"
    GQ = int(os.environ.get("K_GQ", "4"))
    XQ_ACT = os.environ.get("K_XQ", "act") == "act"
    DEBUG = os.environ.get("K_DEBUG", "0") == "1"

    def dbg(name, src_ap):
        if DEBUG and name in t_:
            nc.sync.dma_start(t_[name], src_ap)

    f32 = mybir.dt.float32
    bf16 = mybir.dt.bfloat16
    EXP = mybir.ActivationFunctionType.Exp
    COPY = mybir.ActivationFunctionType.Copy
    MULT = mybir.AluOpType.mult
    ADD = mybir.AluOpType.add
    X = mybir.AxisListType.X
    ts = bass.ts

    with ExitStack() as ctx:
        const = ctx.enter_context(tc.tile_pool(name="const", bufs=1))
        persist = ctx.enter_context(tc.tile_pool(name="persist", bufs=1))

        # ---- constants (SP queue)
        ident = const.tile([128, 128], bf16)
        cmasks.make_identity(nc, ident[:])
        ones_t = const.tile([HD + 1, HD], bf16)
        nc.vector.memset(ones_t[:], 1.0)

        # wqkv first: stage A blocks on it
        wqkv_sb = const.tile([128, 8, 384], bf16)
        nc.sync.dma_start(wqkv_sb[:], t_["wqkv"])
        biasb_sb = const.tile([128, 384], f32)
        if not bias_zero:
            nc.sync.dma_start(biasb_sb[:], t_["biasb"])
        mask_sb = {
            0: const.tile([128, 2048], bf16, name="mc0"),
            1: const.tile([128, 384], bf16, name="mc1"),
            2: const.tile([128, 386], bf16, name="mcI"),
            3: const.tile([128, 258], bf16, name="mc15"),
        }
        for k, nm in ((0, "mc0"), (1, "mc1"), (2, "mcI"), (3, "mc15")):
            nc.sync.dma_start(mask_sb[k][:], t_[nm])
        wrand_sb = const.tile([128, NCH], bf16)
        nc.sync.dma_start(wrand_sb[:], t_["wrand"])
        gidx_sb = const.tile([128, NIDX // 16], mybir.dt.int16)
        nc.sync.dma_start(gidx_sb[:], t_["gidx"])
        wo2_sb = const.tile([HD2, D], bf16)
        nc.sync.dma_start(wo2_sb[:], t_["wo2"])

        # ---- persistent activations
        qrows = persist.tile([128, NRB, 128], bf16)        # row-major q
        qT = persist.tile([128, R], bf16)
        kT = persist.tile([128, R], bf16)
        vaug = [[persist.tile([128, NT, HD + 1], bf16, name=f"vaug{b}{h}")
                 for h in range(HPC)] for b in range(B)]
        # h0's context lives in hout2[0:64]; h1 normalizes into a base-0
        # scratch then DMA-copies across partitions into hout2[64:128]
        hout2 = persist.tile([128, R], bf16)
        hout1 = persist.tile([HD, R], bf16)
        den_sb = persist.tile([HD + 1, 2048], bf16)        # row 64 only
        rinv = persist.tile([HD, 2048], f32)
        kselp = ctx.enter_context(tc.tile_pool(name="kselp", bufs=1))
        ksel = kselp.tile([128, NCH, KVROW], bf16)         # gathered KV rows

        # ---- stage A: fused row-major QKV projection (pairs of row blocks)
        with tc.tile_pool(name="kvrows", bufs=1) as kvpool:
            kvrows = kvpool.tile([128, NRB, 256], bf16)    # row-major k|v
            with tc.tile_pool(name="xstream", bufs=6) as xpool, \
                    tc.tile_pool(name="apsum", bufs=3, space="PSUM") as apsum:
                for pr in range(NRB // 2):
                    xt = xpool.tile([128, 2, 8, 128], bf16, tag="xt")
                    xeng = nc.scalar if XQ_ACT else nc.sync
                    xeng.dma_start(
                        xt[:], t_["xprep"][:, 2 * pr:2 * pr + 2, :, :])
                    # pad to 512/f32 so each j's output stays in one bank
                    ps = apsum.tile([128, 2, 512], f32, tag="ps")
                    for j in range(2):
                        for dc in range(8):
                            nc.tensor.matmul(ps[:, j, 0:384],
                                             xt[:, j, dc, :],
                                             wqkv_sb[:, dc, :],
                                             start=(dc == 0), stop=(dc == 7))
                    rb = 2 * pr
                    if bias_zero:
                        if pr % 2 == 0:
                            nc.scalar.activation(qrows[:, rb:rb + 2, :],
                                                 ps[:, :, 0:128], COPY)
                            nc.scalar.activation(kvrows[:, rb:rb + 2, :],
                                                 ps[:, :, 128:384], COPY)
                        else:
                            nc.vector.tensor_copy(qrows[:, rb:rb + 2, :],
                                                  ps[:, :, 0:128])
                            nc.vector.tensor_copy(kvrows[:, rb:rb + 2, :],
                                                  ps[:, :, 128:384])
                    else:
                        nc.vector.tensor_tensor(
                            qrows[:, rb:rb + 2, :], ps[:, :, 0:128],
                            biasb_sb[:, 0:128].unsqueeze(1)
                            .broadcast_to([128, 2, 128]), op=ADD)
                        nc.vector.tensor_tensor(
                            kvrows[:, rb:rb + 2, :], ps[:, :, 128:384],
                            biasb_sb[:, 128:384].unsqueeze(1)
                            .broadcast_to([128, 2, 256]), op=ADD)
                    b = rb // NT
                    r0 = (rb % NT) * 128
                    nc.sync.dma_start(
                        t_["kv_stage"][r0:r0 + 256, b * 256:(b + 1) * 256]
                        .rearrange("(r p) c -> p r c", p=128),
                        kvrows[:, rb:rb + 2, :])

            # ---- gathers: 6 row-mode SWDGE calls across GQ queues
            for u in range(NIDX // 1024):
                nc.gpsimd.dma_gather(
                    ksel[:, u * 8:(u + 1) * 8, :], t_["kv_stage"],
                    gidx_sb[:, u * 64:(u + 1) * 64],
                    1024, 1024, KVROW, transpose=False, queue_num=u % GQ)

            # ---- qT / kT transposes
            if TRANS_PE:
                with tc.tile_pool(name="tpsum", bufs=4,
                                  space="PSUM") as tpsum:
                    for rb in range(NRB):
                        ptq = tpsum.tile([128, 128], bf16, tag="tp")
                        nc.tensor.transpose(ptq[:], qrows[:, rb, :], ident[:])
                        ptk = tpsum.tile([128, 128], bf16, tag="tp")
                        nc.tensor.transpose(ptk[:], kvrows[:, rb, 0:128],
                                            ident[:])
                        if rb % 2 == 0:
                            nc.scalar.activation(qT[:, ts(rb, 128)], ptq[:],
                                                 COPY)
                            nc.vector.tensor_copy(kT[:, ts(rb, 128)], ptk[:])
                        else:
                            nc.vector.tensor_copy(qT[:, ts(rb, 128)], ptq[:])
                            nc.scalar.activation(kT[:, ts(rb, 128)], ptk[:],
                                                 COPY)
            else:
                for rb in range(NRB):
                    nc.sync.dma_start_transpose(qT[:, ts(rb, 128)],
                                                qrows[:, rb, :])
                    nc.sync.dma_start_transpose(kT[:, ts(rb, 128)],
                                                kvrows[:, rb, 0:128])

            # ---- vaug: V rows + ones rider
            for b in range(B):
                for h in range(HPC):
                    nc.vector.tensor_copy(
                        vaug[b][h][:, :, 0:HD],
                        kvrows[:, b * NT:(b + 1) * NT,
                               128 + HD * h:128 + HD * (h + 1)])
                    nc.vector.memset(vaug[b][h][:, :, HD:HD + 1], 1.0)

            dbg("dbg_qrows", qrows[:])
            dbg("dbg_kvrows", kvrows[:])
            dbg("dbg_qT", qT[:])
            dbg("dbg_kT", kT[:])
            dbg("dbg_ksel", ksel[:])

        # ---- attention
        BH = [(b, h) for b in range(B) for h in range(HPC)]

        with tc.tile_pool(name="pj", bufs=2) as pjpool, \
                tc.tile_pool(name="rsc", bufs=1) as rpool, \
                tc.tile_pool(name="wsb", bufs=2) as wpool, \
                tc.tile_pool(name="nrm", bufs=2) as npool, \
                tc.tile_pool(name="osb", bufs=3) as opool, \
                tc.tile_pool(name="ctx", bufs=1, space="PSUM") as ctxpsum, \
                tc.tile_pool(name="s512", bufs=2, space="PSUM") as spsum, \
                tc.tile_pool(name="bc", bufs=2, space="PSUM") as bcpsum:

            pj_t = {}
            wsumb_t = {}

            def qk_phase(b, h):
                hs = slice(HD * h, HD * (h + 1))
                pj = pjpool.tile([128, 8192], bf16, tag="pj",
                                 name=f"pj{b}{h}")
                pj_t[(b, h)] = pj
                col = 0
                for jb in range(NT):
                    i0, w, extra = _serve(jb)
                    lhsT = kT[hs, b * S + jb * 128:b * S + (jb + 1) * 128]
                    if jb == 0:
                        for seg in range(4):
                            psc = spsum.tile([128, 512], f32, tag="psc")
                            nc.tensor.matmul(
                                psc[:], lhsT,
                                qT[hs, b * S + seg * 512:
                                   b * S + (seg + 1) * 512],
                                start=True, stop=not MASK_MM)
                            if MASK_MM:
                                nc.tensor.matmul(
                                    psc[:], ident[:],
                                    mask_sb[0][:, ts(seg, 512)],
                                    start=False, stop=True)
                            else:
                                nc.vector.tensor_tensor(
                                    psc[:], psc[:],
                                    mask_sb[0][:, ts(seg, 512)], op=ADD)
                            nc.scalar.activation(
                                pj[:, seg * 512:(seg + 1) * 512], psc[:], EXP)
                        col = 2048
                        continue
                    tot = w + (2 if extra else 0)
                    psc = spsum.tile([128, 512], f32, tag="psc")
                    nc.tensor.matmul(psc[:, 0:w], lhsT,
                                     qT[hs, b * S + i0:b * S + i0 + w],
                                     start=True,
                                     stop=(not MASK_MM and not extra))
                    if extra:
                        # start=False: the band matmul's bank-clear already
                        # reset has_written for this region (start=True here
                        # would wipe the band scores' bits bank-wide)
                        nc.tensor.matmul(psc[:, w:w + 2], lhsT,
                                         qT[hs, b * S:b * S + 2],
                                         start=False, stop=not MASK_MM)
                    if MASK_MM:
                        nc.tensor.matmul(psc[:, 0:tot], ident[:],
                                         mask_sb[_mask_kind(jb)][:, 0:tot],
                                         start=False, stop=True)
                    else:
                        nc.vector.tensor_tensor(
                            psc[:, 0:tot], psc[:, 0:tot],
                            mask_sb[_mask_kind(jb)][:, 0:tot], op=ADD)
                    nc.scalar.activation(pj[:, col:col + tot],
                                         psc[:, 0:tot], EXP)
                    col += tot
                if (b, h) == (0, 0):
                    dbg("dbg_pj", pj[:])

            def rand_prep(b, h):
                dotp = rpool.tile([128, NT, NR, HD], bf16, tag="dotp")
                kv_k = ksel[:, :, b * 256 + HD * h:b * 256 + HD * (h + 1)]
                kv_k = kv_k.rearrange("p (t m) c -> p t m c", t=NT)
                qrep = qrows[:, b * NT:(b + 1) * NT, HD * h:HD * (h + 1)]
                qrep = qrep.unsqueeze(2).broadcast_to([128, NT, NR, HD])
                nc.vector.tensor_tensor(dotp[:], kv_k, qrep, op=MULT)
                sval = rpool.tile([128, NCH], f32, tag="sval")
                nc.vector.tensor_reduce(
                    sval[:].rearrange("p (t m) -> p t m", t=NT), dotp[:],
                    axis=X, op=ADD)
                coef = rpool.tile([128, NCH], f32, tag="coef")
                nc.scalar.activation(coef[:], sval[:], EXP)
                nc.vector.tensor_tensor(coef[:], coef[:], wrand_sb[:],
                                        op=MULT)
                # reuse dotp as the weighted-V scratch (sequential WAR)
                kv_v = ksel[:, :, b * 256 + 128 + HD * h:
                            b * 256 + 128 + HD * (h + 1)]
                kv_v = kv_v.rearrange("p (t m) c -> p t m c", t=NT)
                crep = coef[:].rearrange("p (t m) -> p t m", t=NT)
                crep = crep.unsqueeze(3).broadcast_to([128, NT, NR, HD])
                nc.vector.tensor_tensor(dotp[:], kv_v, crep, op=MULT)
                wsumf = rpool.tile([128, NT, HD + 1], f32, tag="wsumf")
                nc.vector.tensor_reduce(
                    wsumf[:, :, 0:HD],
                    dotp[:].rearrange("p t m c -> p t c m"), axis=X, op=ADD)
                nc.vector.tensor_reduce(
                    wsumf[:, :, HD:HD + 1],
                    coef[:].rearrange("p (t m) -> p t m", t=NT),
                    axis=X, op=ADD)
                wsumb = wpool.tile([128, NT, HD + 1], bf16, tag="wsumb",
                                   name=f"wsumb{b}{h}")
                wsumb_t[(b, h)] = wsumb
                nc.vector.tensor_copy(wsumb[:], wsumf[:])
                if (b, h) == (0, 0):
                    dbg("dbg_wsumf", wsumf[:])

            def pv_phase(b, h):
                pj = pj_t[(b, h)]
                wsumb = wsumb_t[(b, h)]
                ctxp = ctxpsum.tile([128, 2048], f32, tag="ctx")
                col = 0
                for jb in range(NT):
                    i0, w, extra = _serve(jb)
                    lhsT = vaug[b][h][:, jb, :]
                    if jb == 0:
                        for seg in range(4):
                            nc.tensor.matmul(
                                ctxp[0:HD + 1, ts(seg, 512)], lhsT,
                                pj[:, ts(seg, 512)], start=True, stop=False)
                        col = 2048
                        continue
                    # split at 512-col PSUM bank boundaries
                    a = i0
                    while a < i0 + w:
                        e = min(i0 + w, (a // 512 + 1) * 512)
                        nc.tensor.matmul(
                            ctxp[0:HD + 1, a:e], lhsT,
                            pj[:, col + (a - i0):col + (e - i0)],
                            start=False, stop=False)
                        a = e
                    if extra:
                        nc.tensor.matmul(ctxp[0:HD + 1, 0:2], lhsT,
                                         pj[:, col + w:col + w + 2],
                                         start=False, stop=False)
                    col += w + (2 if extra else 0)
                for t in range(NT):
                    nc.tensor.matmul(ctxp[0:HD + 1, ts(t, 128)],
                                     wsumb[:, t, :], ident[:],
                                     start=False, stop=True)
                # normalize: den row 64 -> bcast matmul -> 1/x -> mult
                for seg in range(4):
                    nc.scalar.activation(den_sb[HD:HD + 1, ts(seg, 512)],
                                         ctxp[HD:HD + 1, ts(seg, 512)], COPY)
                    bcp = bcpsum.tile([HD, 512], f32, tag="bc")
                    nc.tensor.matmul(bcp[:], ones_t[HD:HD + 1, :],
                                     den_sb[HD:HD + 1, ts(seg, 512)],
                                     start=True, stop=True)
                    if RECIP_FAST:
                        # approx-fast misreads PSUM sources: stage via SBUF
                        bsb = npool.tile([HD, 512], f32, tag="bsb")
                        nc.scalar.activation(bsb[:], bcp[:], COPY)
                        nc.vector.reciprocal_approx_fast(
                            rinv[:, ts(seg, 512)], bsb[:])
                    else:
                        nc.vector.reciprocal(rinv[:, ts(seg, 512)], bcp[:])
                    dst = hout2[0:HD, :] if h == 0 else hout1[:]
                    nc.vector.tensor_tensor(
                        dst[:, b * S + seg * 512:b * S + (seg + 1) * 512],
                        ctxp[0:HD, ts(seg, 512)], rinv[:, ts(seg, 512)],
                        op=MULT)
                if h == 1:
                    # cross-partition merge (only DMA can change partitions)
                    nc.sync.dma_start(
                        hout2[HD:HD2, b * S:(b + 1) * S],
                        hout1[:, b * S:(b + 1) * S])
                if (b, h) == (0, 0):
                    dbg("dbg_den", den_sb[:])
                    dbg("dbg_rinv", rinv[:])

            def d_stage(b):
                # partial o-projection for batch b (interleaves attention)
                for rc in range(b * NT, (b + 1) * NT):
                    ob = opool.tile([128, D], bf16, tag="ob")
                    for half in range(2):
                        po = spsum.tile([128, 512], f32, tag="psc")
                        nc.tensor.matmul(po[:], hout2[:, ts(rc, 128)],
                                         wo2_sb[:, ts(half, 512)],
                                         start=True, stop=True)
                        if (rc * 2 + half) % 2 == 0:
                            nc.vector.tensor_copy(ob[:, ts(half, 512)], po[:])
                        else:
                            nc.scalar.activation(ob[:, ts(half, 512)], po[:],
                                                 COPY)
                    nc.sync.dma_start(t_["out"][ts(rc, 128), :], ob[:])

            # software pipeline: QK one bh ahead of PV; stage D per batch
            # fills PE gaps.  Scalar-engine queue is strict FIFO, so emit
            # scalar work (exps) in dependency-ready order.
            qk_phase(*BH[0])
            qk_phase(*BH[1])
            rand_prep(*BH[0])
            pv_phase(*BH[0])
            qk_phase(*BH[2])
            rand_prep(*BH[1])
            pv_phase(*BH[1])
            d_stage(0)
            qk_phase(*BH[3])
            rand_prep(*BH[2])
            pv_phase(*BH[2])
            rand_prep(*BH[3])
            pv_phase(*BH[3])
            d_stage(1)
            dbg("dbg_hout0", hout2[0:HD, :])
            dbg("dbg_hout1", hout1[:])


# ---------------------------------------------------------------- execution
_NC_CACHE = {}


def _get_nc(bias_zero=True):
    if bias_zero not in _NC_CACHE:
        _NC_CACHE[bias_zero] = build_kernel(bias_zero)
    return _NC_CACHE[bias_zero]


def _install_axon_trace_shim():
    import sys
    import types

    if "antenv.axon_hooks" in sys.modules:
        return
    mod = types.ModuleType("antenv.axon_hooks")
    mod._hook = None
    mod.set_axon_ntff_profile_hook = lambda h: setattr(mod, "_hook", h)
    mod.get_axon_ntff_profile_hook = lambda: mod._hook
    sys.modules["antenv.axon_hooks"] = mod
    try:
        import antenv
        antenv.axon_hooks = mod
        from trn_agent_boot.trn_boot import _ntff_profile_via_ctypes
        mod._hook = _ntff_profile_via_ctypes("/opt/axon/libaxon_pjrt.so")
    except Exception:
        pass


def run_on_hw(in_maps, trace=False, trace_kwargs=None, bias_zero=True):
    """Compile+run on the 8 cores; returns (results, BassKernelResults)."""
    _install_axon_trace_shim()
    from concourse import bass_utils
    bass_utils.upload_artifacts = lambda tmpdir: f"local:{tmpdir}"

    nc = _get_nc(bias_zero)
    res = bass_utils.run_bass_kernel_spmd(
        nc, in_maps, core_ids=list(range(N_CORES)), trace=trace,
        trace_kwargs=trace_kwargs or {})
    return res.results, res


def _bias_zero(inputs):
    return all(
        not np.any(np.asarray(inputs[k], dtype=np.float32))
        for k in ("q_b", "k_b", "v_b"))


def kernel(**inputs):
    in_maps = make_in_maps(inputs)
    results, _ = run_on_hw(in_maps, trace=False, bias_zero=_bias_zero(inputs))
    out = np.zeros((R, D), dtype=np.float32)
    for c in range(N_CORES):
        out += np.asarray(results[c]["out_part"], dtype=np.float32)
    out += np.asarray(inputs["o_b"], dtype=np.float32)[None, :]
    return out.reshape(B, S, D)
